# revision 44
# baseline (speedup 1.0000x reference)
import sys
sys.path.insert(0, '/opt/trn_rl_repo')
import numpy as np
import ml_dtypes

BF16 = ml_dtypes.bfloat16

N = 25000
E = 400000
NCORES = 8
GB = 4               # edge tiles fetched per ga DMA

_CACHE = {}


def _prep_weights(W_sc_s, W_sc_v, W1_s, W1_v, W_r1, W_r2, W2_s, W2_v):
    """Host-side weight folding.

    Feature layout (f, 160): [s(64) | vx(32) | vy(32) | vz(32)]  (c-major v)
    mid column layout (640): b-major blocks of 160, b=0: attr se, b=1..3:
    attr ve_{b-1}; within a block the f layout above.
    """
    c_s, c_x = np.sin(np.pi / 8.0), np.cos(np.pi / 8.0)
    # lin1 (host): x rows [s|vx|vy|vz] -> g cols [s1|v1x|v1y|v1z]
    Wnode = np.zeros((160, 160), np.float32)
    Wnode[0:64, 0:64] = W1_s / 8.0
    for c in range(3):
        Wnode[64 + 32 * c:96 + 32 * c, 64 + 32 * c:96 + 32 * c] = W1_v / np.sqrt(32.0)
    # self-connection (host): -> [y_s-pre(96) | y_v-pre c-major(96)], c_s folded
    Wsc = np.zeros((160, 192), np.float32)
    Wsc[0:64, 0:96] = W_sc_s / 8.0 * c_s
    for c in range(3):
        Wsc[64 + 32 * c:96 + 32 * c, 96 + 32 * c:128 + 32 * c] = \
            W_sc_v / np.sqrt(32.0) * c_s
    Wr1p = (W_r1 / np.sqrt(12.0)).astype(np.float32)
    # radial-2: [100, 640], b-major blocks of 160 = [w1/w2(64)|32|32|32]
    w1 = W_r2[:, 0:64] / 10.0
    w2 = W_r2[:, 64:128] / 10.0
    w3 = W_r2[:, 128:160] / 10.0
    w4 = W_r2[:, 160:192] / 10.0
    w5 = W_r2[:, 192:224] / 10.0
    Wr2p = np.zeros((100, 640), np.float32)
    Wr2p[:, 0:64] = w1
    for c in range(3):
        Wr2p[:, 64 + 32 * c:96 + 32 * c] = w3
    for b in range(1, 4):
        o = 160 * b
        Wr2p[:, o:o + 64] = w2
        for cp in range(3):
            Wr2p[:, o + 64 + 32 * cp:o + 96 + 32 * cp] = w4 if cp == b - 1 else w5
    # lin2 over mid(640) -> yp cols [y_s(96) | y_v c-major(96)]
    k = c_x / 4.0
    ks = k / np.sqrt(96.0)
    kv = k / np.sqrt(128.0)
    eps = np.zeros((3, 3, 3), np.float32)
    eps[0, 1, 2] = eps[1, 2, 0] = eps[2, 0, 1] = 1.0
    eps[0, 2, 1] = eps[1, 0, 2] = eps[2, 1, 0] = -1.0
    W2p = np.zeros((640, 192), np.float32)
    W2p[0:64, 0:96] = W2_s[0:64] * ks                       # m0a
    for c in range(3):
        W2p[64 + 32 * c:96 + 32 * c, 96 + 32 * c:128 + 32 * c] = W2_v[64:96] * kv  # m1b
    for c in range(3):                                      # attr = ve_c
        o = 160 * (c + 1)
        W2p[o:o + 64, 96 + 32 * c:128 + 32 * c] = W2_v[0:64] * kv                  # m1a
        for cp in range(3):
            r = o + 64 + 32 * cp
            if cp == c:
                W2p[r:r + 32, 0:96] = W2_s[64:96] * ks / np.sqrt(3.0)              # m0b
            else:
                i = 3 - c - cp
                sgn = eps[i, cp, c]
                W2p[r:r + 32, 96 + 32 * i:128 + 32 * i] = \
                    W2_v[96:128] * kv * sgn / np.sqrt(2.0)                          # m1c
    return (Wnode, Wsc, Wr1p.astype(BF16), Wr2p.astype(BF16), W2p)


def _assign_slots(edge_dst, NWIN):
    """Bin nodes into 8 cores x NWIN windows x 128 slots, greedily balancing
    edge count per window (nodes in degree-descending order)."""
    NW = NCORES * NWIN
    deg = np.bincount(edge_dst, minlength=N)
    order = np.argsort(-deg, kind='stable')
    wsum = np.zeros(NW, np.int64)
    wcnt = np.zeros(NW, np.int64)
    core = np.empty(N, np.int64)
    slot = np.empty(N, np.int64)
    BIG = 1 << 40
    for n in order:
        w = int(np.argmin(np.where(wcnt < 128, wsum, BIG)))
        core[n] = w // NWIN
        slot[n] = (w % NWIN) * 128 + wcnt[w]
        wsum[w] += deg[n]
        wcnt[w] += 1
    return core, slot, wsum


def _prep_core(c, g, edge_src, edge_dst, edge_attr, edge_scalars, NWIN, WT,
               core, slot):
    sel = np.nonzero(core[edge_dst] == c)[0]
    eslot = slot[edge_dst[sel]]
    win = eslot >> 7
    order = np.argsort(win, kind='stable')
    sel = sel[order]
    eslot = eslot[order]
    win = win[order]

    TW = WT * 128
    EP = NWIN * TW
    gaT = np.zeros((EP, 640), BF16)
    es_p = np.zeros((EP, 12), np.float32)
    col_p = np.full(EP, -1.0, np.float32)
    for w in range(NWIN):
        m = win == w
        ew = sel[m]
        k = ew.size
        o = w * TW
        # ga rows: ea[e, b] * g[src[e], f], b-major 640 cols
        gg = g[edge_src[ew]]                       # [k,160]
        ea = edge_attr[ew]                         # [k,4]
        gaT[o:o + k] = (ea[:, :, None] * gg[:, None, :]).reshape(k, 640).astype(BF16)
        es_p[o:o + k] = edge_scalars[ew]
        col_p[o:o + k] = (eslot[m] & 127).astype(np.float32)

    # repack ga for batched DMA: per window, chunks of GB tiles laid out
    # partition-major ([128, cl*640] per chunk, contiguous per partition)
    for w in range(NWIN):
        for t0 in range(0, WT, GB):
            cl = min(GB, WT - t0)
            r0 = (w * WT + t0) * 128
            blk = gaT[r0:r0 + cl * 128].reshape(cl, 128, 640)
            gaT[r0:r0 + cl * 128] = np.ascontiguousarray(
                blk.transpose(1, 0, 2)).reshape(cl * 128, 640)

    T = EP // 128
    esT = np.ascontiguousarray(es_p.T).astype(BF16)
    dstT = np.ascontiguousarray(col_p.reshape(T, 128).T)
    return dict(gaT=gaT, esT=esT, dstT=dstT)


def _build_program(NWIN, WT):
    import concourse.bass as bass
    import concourse.tile as tile
    from concourse import bacc, mybir

    f32 = mybir.dt.float32
    bf16 = mybir.dt.bfloat16
    i32 = mybir.dt.int32
    AF = mybir.ActivationFunctionType
    MUL = mybir.AluOpType.mult
    EQ = mybir.AluOpType.is_equal
    TW = WT * 128
    EP = NWIN * TW
    NPC = NWIN * 128

    nc = bacc.Bacc("TRN2", num_devices=NCORES, debug=False)
    gaT_ap = nc.dram_tensor("gaT", [EP, 640], bf16, kind="ExternalInput").ap()
    esT_ap = nc.dram_tensor("esT", [12, EP], bf16, kind="ExternalInput").ap()
    dstT_ap = nc.dram_tensor("dstT", [128, EP // 128], f32,
                             kind="ExternalInput").ap()
    Wr1_ap = nc.dram_tensor("Wr1p", [12, 100], bf16, kind="ExternalInput").ap()
    Wr2_ap = nc.dram_tensor("Wr2p", [100, 640], bf16, kind="ExternalInput").ap()
    out_ap = nc.dram_tensor("out", [NPC, 640], bf16, kind="ExternalOutput").ap()

    with tile.TileContext(nc) as tc:
        from contextlib import ExitStack
        with ExitStack() as ctx:
            wpool = ctx.enter_context(tc.tile_pool(name="weights", bufs=1))

            wr1 = wpool.tile([12, 100], bf16)
            wr2 = wpool.tile([100, 640], bf16)
            nc.sync.dma_start(wr1[:], Wr1_ap[:])
            nc.sync.dma_start(wr2[:], Wr2_ap[:])

            ioti = wpool.tile([128, 128], i32)
            iot = wpool.tile([128, 128], bf16)
            nc.gpsimd.iota(ioti[:], pattern=[[1, 128]], base=0,
                           channel_multiplier=0)
            nc.vector.tensor_copy(iot[:], ioti[:])

            esP = ctx.enter_context(tc.tile_pool(name="esw", bufs=4))
            hcP = ctx.enter_context(tc.tile_pool(name="hc", bufs=3))
            hpP = ctx.enter_context(tc.tile_pool(name="hp", bufs=1, space="PSUM"))
            gaP = ctx.enter_context(tc.tile_pool(name="ga", bufs=5))
            waP = ctx.enter_context(tc.tile_pool(name="wpa", bufs=3, space="PSUM"))
            wbP = ctx.enter_context(tc.tile_pool(name="wpb", bufs=2, space="PSUM"))
            wbsP = ctx.enter_context(tc.tile_pool(name="wpbs", bufs=5))
            midP = ctx.enter_context(tc.tile_pool(name="mid", bufs=6))
            ohP = ctx.enter_context(tc.tile_pool(name="oh", bufs=10))
            acAP = ctx.enter_context(tc.tile_pool(name="acc0", bufs=1, space="PSUM"))
            acBP = ctx.enter_context(tc.tile_pool(name="acc1", bufs=1, space="PSUM"))
            dsP = ctx.enter_context(tc.tile_pool(name="dsw", bufs=2))
            csbP = ctx.enter_context(tc.tile_pool(name="csb", bufs=2))

            st_acc = {}
            st_es = {}
            st_hc = {}
            RCH = (TW + 511) // 512  # radial 512-col chunks per window

            def emit_es(w):
                if w >= NWIN:
                    return
                esw = esP.tile([12, TW], bf16, tag="esw", name="esw")
                nc.sync.dma_start(esw[:], esT_ap[:, w * TW:(w + 1) * TW])
                st_es[w] = esw

            def emit_radial_chunk(w, c):
                # one 512-col radial chunk of window w (silu shares the act
                # table with Copy, so these interleave freely with copies)
                if w >= NWIN:
                    return
                if c == 0:
                    st_hc[w] = hcP.tile([100, TW], bf16, tag="hc", name="hc")
                hc = st_hc[w]
                esw = st_es[w]
                cw = min(512, TW - c * 512)
                hp = hpP.tile([100, 512], f32, tag="hp", name="hp")
                nc.tensor.matmul(hp[:, 0:cw], wr1[:],
                                 esw[:, c * 512:c * 512 + cw],
                                 start=True, stop=True)
                nc.scalar.activation(hc[:, c * 512:c * 512 + cw], hp[:, 0:cw],
                                     AF.Silu)
                if c == RCH - 1:
                    st_es.pop(w)

            def emit_csb(w):
                # node accumulators PSUM -> SBUF bf16, then straight to HBM;
                # lin2 + gate happen on the host
                acc0, acc1 = st_acc.pop(w)
                csb = csbP.tile([128, 640], bf16, tag="csb", name="csb")
                nc.scalar.activation(csb[:, 0:320], acc0[:], AF.Copy)
                nc.scalar.activation(csb[:, 320:640], acc1[:], AF.Copy)
                nc.sync.dma_start(out_ap[w * 128:(w + 1) * 128, :], csb[:])

            wpa_tiles = {}

            def emit_wpa(w, t):
                wpa = waP.tile([128, 512], f32, tag="wpa", name="wpa")
                wpb = wbP.tile([128, 128], f32, tag="wpb", name="wpb")
                hsl = st_hc[w][:, t * 128:(t + 1) * 128]
                nc.tensor.matmul(wpa[:], hsl, wr2[:, 0:512],
                                 start=True, stop=True)
                nc.tensor.matmul(wpb[:], hsl, wr2[:, 512:640],
                                 start=True, stop=True)
                # GPSIMD cannot read PSUM: stage the 128-col half through
                # SBUF via the Activation engine so Pool can multiply it
                wpbs = wbsP.tile([128, 128], bf16, tag="wpbs", name="wpbs")
                nc.scalar.activation(wpbs[:], wpb[:], AF.Copy)
                wpa_tiles[t] = (wpa, wpbs)

            def emit_chunk_dma(w, t0):
                cl = min(GB, WT - t0)
                gac = gaP.tile([128, cl * 640], bf16, tag="ga", name="gac")
                r0 = (w * WT + t0) * 128
                src = gaT_ap[r0:r0 + cl * 128, :].rearrange(
                    "(p k) f -> p (k f)", p=128)
                nc.sync.dma_start(gac[:], src)
                return gac

            pend_acc = []

            def emit_acc(acc0, acc1, t, mid, oh):
                st = (t == 0)
                sp = (t == WT - 1)
                nc.tensor.matmul(acc0[:], oh[:], mid[:, 0:320],
                                 start=st, stop=sp)
                nc.tensor.matmul(acc1[:], oh[:], mid[:, 320:640],
                                 start=st, stop=sp)

            def emit_chunk(w, t0, gac, dsw, acc0, acc1):
                cl = min(GB, WT - t0)
                ohs = []
                for dt in range(cl):
                    oh = ohP.tile([128, 128], bf16, tag="oh", name="oh")
                    nc.gpsimd.tensor_scalar(oh[:], iot[:],
                                            dsw[:, t0 + dt:t0 + dt + 1],
                                            None, op0=EQ)
                    ohs.append(oh)
                for dt in range(cl):
                    t = t0 + dt
                    ga = gac[:, dt * 640:(dt + 1) * 640]
                    wpa, wpbs = wpa_tiles.pop(t)
                    mid = midP.tile([128, 640], bf16, name="mid")
                    nc.vector.tensor_tensor(mid[:, 0:512], wpa[:],
                                            ga[:, 0:512], MUL)
                    nc.gpsimd.tensor_tensor(mid[:, 512:640], wpbs[:],
                                            ga[:, 512:640], MUL)
                    if t + 2 < WT:
                        emit_wpa(w, t + 2)
                    if t % 4 == 0:
                        emit_radial_chunk(w + 2, t // 4)
                    # lag acc matmuls 2 tiles so PE does not block on the
                    # prior window csb drain (acc buffer WAR, bufs=1)
                    pend_acc.append((acc0, acc1, t, mid, ohs[dt]))
                    if len(pend_acc) > 2:
                        emit_acc(*pend_acc.pop(0))
                    if t == WT - 1:
                        while pend_acc:
                            emit_acc(*pend_acc.pop(0))

            # bootstrap: radial for the first two windows up front
            emit_es(0)
            emit_es(1)
            for c in range(RCH):
                emit_radial_chunk(0, c)
            for c in range(RCH):
                emit_radial_chunk(1, c)

            nchunks = (WT + GB - 1) // GB
            for w in range(NWIN):
                emit_es(w + 2)
                gacs = {0: emit_chunk_dma(w, 0)}
                if 1 < nchunks:
                    gacs[1] = emit_chunk_dma(w, GB)
                dsw = dsP.tile([128, WT], f32, tag="dsw", name="dsw")
                nc.sync.dma_start(dsw[:], dstT_ap[:, w * WT:(w + 1) * WT])
                emit_wpa(w, 0)
                emit_wpa(w, 1)
                if w - 1 >= 0:
                    emit_csb(w - 1)
                acc0 = acAP.tile([128, 320], f32, tag="acc0", name="acc0")
                acc1 = acBP.tile([128, 320], f32, tag="acc1", name="acc1")
                st_acc[w] = (acc0, acc1)
                for ci, t0 in enumerate(range(0, WT, GB)):
                    if ci + 2 < nchunks:
                        gacs[ci + 2] = emit_chunk_dma(w, (ci + 2) * GB)
                    emit_chunk(w, t0, gacs.pop(ci), dsw, acc0, acc1)
                st_hc.pop(w)

            emit_csb(NWIN - 1)

    nc.compile()
    return nc


def kernel(x, z, edge_src, edge_dst, edge_attr, edge_scalars,
           W_sc_s, W_sc_v, W1_s, W1_v, W_r1, W_r2, W2_s, W2_v):
    from concourse import bass_utils
    x = np.asarray(x, np.float32)
    z = np.asarray(z, np.float32)
    edge_src = np.asarray(edge_src, np.int64)
    edge_dst = np.asarray(edge_dst, np.int64)
    edge_attr = np.asarray(edge_attr, np.float32)
    edge_scalars = np.asarray(edge_scalars, np.float32)

    # pick the window count minimizing total edge tiles (tie: fewer windows)
    best = None
    for nwin in (26, 27, 28, 30):
        core_, slot_, wsum_ = _assign_slots(edge_dst, nwin)
        wt_ = int(np.ceil(wsum_.max() / 128.0))
        cand = (nwin * wt_, nwin, wt_, core_, slot_)
        if best is None or cand[0] < best[0]:
            best = cand
    _, NWIN, WT, core, slot = best

    key = (NWIN, WT)
    if key not in _CACHE:
        _CACHE[key] = _build_program(NWIN, WT)
    nc = _CACHE[key]

    Wnode, Wsc, Wr1p, Wr2p, W2p = _prep_weights(
        np.asarray(W_sc_s, np.float32), np.asarray(W_sc_v, np.float32),
        np.asarray(W1_s, np.float32), np.asarray(W1_v, np.float32),
        np.asarray(W_r1, np.float32), np.asarray(W_r2, np.float32),
        np.asarray(W2_s, np.float32), np.asarray(W2_v, np.float32))

    # host-side lin1 / self-connection (x feature cols -> c-major layout)
    xrow = np.concatenate([np.arange(64), 64 + 3 * np.arange(32),
                           65 + 3 * np.arange(32), 66 + 3 * np.arange(32)])
    x2 = (x * z)[:, xrow]
    g = x2 @ Wnode                                  # [N,160]
    scH = x2 @ Wsc                                  # [N,192]

    in_maps = []
    for c in range(NCORES):
        m = _prep_core(c, g, edge_src, edge_dst, edge_attr, edge_scalars,
                       NWIN, WT, core, slot)
        m.update(Wr1p=Wr1p, Wr2p=Wr2p)
        in_maps.append(m)

    res = bass_utils.run_bass_kernel_spmd(nc, in_maps, core_ids=list(range(NCORES)))

    # host tail: lin2 + self-connection mix + gate (small: [N,640] @ [640,192])
    out = np.empty((N, 160), np.float32)
    for c in range(NCORES):
        acc = res.results[c]["out"].astype(np.float32)        # [NPC, 640]
        own = np.nonzero(core == c)[0]
        sl = slot[own]
        y2 = acc[sl] @ W2p + scH[own]                         # [n,192]
        sig = 1.0 / (1.0 + np.exp(-y2[:, 0:96]))
        out[own, 0:64] = y2[:, 0:64] * sig[:, 0:64]
        gated = y2[:, 96:192].reshape(-1, 3, 32) * sig[:, None, 64:96]
        out[own, 64:160] = gated.transpose(0, 2, 1).reshape(-1, 96)
    return out
